# revision 16
# baseline (speedup 1.0000x reference)
"""Trainium2 Bass kernel for nn_GCDDLayer (Gaussian-curvature diffusion layer).

Math (per 512x512 image, zero-padded 3x3 convs):
    ux  = conv(u, SOBEL_X);  uy  = conv(u, SOBEL_Y)
    uxx = conv(ux, SOBEL_X); uxy = conv(ux, SOBEL_Y); uyy = conv(uy, SOBEL_Y)
    G   = (uxx*uyy - uxy^2) / ((1 + ux^2 + uy^2)^2 + 1e-6)
    phi = exp(-|G|); P = phi*ux; Q = phi*uy
    out = u + conv(P, SOBEL_X) + conv(Q, SOBEL_Y)

Strategy: pure data parallel over batch (16 samples -> 8 cores x 2 samples),
each core processes 6 independent 512x512 images (2 samples x 3 channels),
each cut into 5 overlapping 128-row tiles (stride 122; 3-row halo absorbs the
3-deep conv chain). Convs run on the TensorEngine as banded-matrix matmuls
(y-direction via the band, x-direction via shifted column reads of zero-padded
SBUF tiles, accumulated in PSUM).

v2 (~140us vs v1's ~226us; measured via the GCDD_REPS slope method):
- bf16 for u, out, and all SBUF intermediates: 2x DVE tensor_tensor mode,
  halved DMA traffic. Verified vs the fp32 jax reference: rel err ~1.0e-2
  (budget 2e-2; fp32 v1 was ~1e-3).
- |num| as a bf16 sign-bit mask: bitcast to int32, tensor_scalar AND with
  0x7FFF7FFF (2x_2p) — replaces an ACT Abs pass.
- the +1 in q = 1+ux^2+uy^2 rides ACT Ln's free bias; the q reduction is a
  plain 2x tensor_add instead of a 1x scalar_tensor_tensor.
- engine balance (HW-swept): all PSUM->SBUF evacs of ux/uy/uxx/uxy on ACT,
  out = u + div adds on DVE, Ln/Exp/Exp on ACT (one table set), everything
  else pointwise on DVE. GPSIMD/Pool measured net-negative on HW for every
  offload tried (sim's 0.42-efficiency model is optimistic) - left off.
- pointwise chunked [[0,1],[1,2],[2,4],[4,5]] so ACT/DVE/PE pipeline within
  an image; the last image uses 1-tile chunks to shorten the drain tail.

Engine busy (CoreSim, per core): ACT ~118us, PE ~117us (540 matmuls), DVE
~108us -> three-way balanced; wall ~137us sim / ~141us HW. Rejected paths:
uxy from conv(uy,SX) (boundary error 5.7e-2), Pool x-derivative precompute,
custom fused DVE ops (walrus in this container can't codegen them), DMA from
PSUM (unsupported), emission skew/interleave (scheduler already optimal).
"""

import os

import numpy as np

B, C, H, W = 16, 3, 512, 512
N_CORES = 8
IMGS = (B // N_CORES) * C  # 6 images per core
# PAD=4 keeps the center views 4B-aligned (DVE 2x_1p needs aligned packed
# reads); 3 is the minimum the conv chain needs.
PAD = int(os.environ.get("GCDD_PAD", "4"))
BLK = W + 2 * PAD
NT = 5  # row tiles per image
TILE_STARTS = [0, 122, 244, 366, 384]
OUT_ROWS = [(0, 125), (125, 247), (247, 369), (369, 491), (491, 512)]
WIDTH = NT * BLK  # 2590
PWIDTH = NT * W  # 2560

_CACHE = {}


def _split_multiwaits(nc):
    """Walrus in this container accepts only one sync-wait per instruction;
    Tile emits multi-wait instructions. Split: for an instruction with k>1
    waits, insert k-1 single-wait NoOps before it on the same engine (engine
    queues are strict FIFO, so sequential waiting is equivalent)."""
    import concourse.mybir as mybir

    ctr = [0]

    def fresh(base):
        ctr[0] += 1
        return f"{base}-wsplit{ctr[0]}"

    for f in nc.m.functions:
        for b in f.blocks:
            changed = False
            newlist = []
            for ins in b.instructions:
                si = ins.sync_info
                if si is not None and len(si.on_wait) > 1:
                    waits = list(si.on_wait)
                    for w in waits[:-1]:
                        newlist.append(
                            mybir.InstNoOp(
                                name=fresh(ins.name),
                                engine=ins.engine,
                                debug=ins.debug,
                                ins=[],
                                outs=[],
                                sync_info=mybir.SyncInfo(on_wait=[w], on_update=[]),
                            )
                        )
                    ins.sync_info = mybir.SyncInfo(
                        on_wait=[waits[-1]], on_update=list(si.on_update)
                    )
                    changed = True
                newlist.append(ins)
            if changed:
                b.instructions = newlist


def _band(c0, c1, c2, n=128):
    # lhsT[k, m] = col[k - m + 1] (k: input row partition, m: output row)
    return (
        np.diag(np.full(n, c1))
        + np.diag(np.full(n - 1, c0), 1)
        + np.diag(np.full(n - 1, c2), -1)
    ).astype(np.float32)


def _bands_np():
    a = _band(1, 2, 1)
    return np.stack(
        [
            a,  # BSp: SOBEL_X col dx=+1 (also the A smoothing band)
            _band(-1, -2, -1),  # BSm: SOBEL_X col dx=-1
            _band(-1, 0, 1),  # BD : SOBEL_Y col dx=+-1
            _band(-2, 0, 2),  # BD2: SOBEL_Y col dx=0
            np.eye(128, dtype=np.float32),  # IDT: residual u
            (a @ a).astype(np.float32),  # A2: y-smooth twice (pentadiagonal)
        ]
    )


def _sq_add_op():
    """Custom DVE op: out = in0^2 + in1^2 (one instruction instead of
    mul+mul+add). Registered at runtime; the uop table ships in the NEFF."""
    import numpy as np

    import concourse.dve_ops as dve_ops
    from concourse.dve_spec import Spec, Src0, Src1, lower, sq
    from concourse.dve_table_gen import dve_ver_for
    from concourse.dve_uop import DveOpSpec

    for op in dve_ops.OPS:
        if op.name == "SQ_ADD_ANT":
            return op
    op = dve_ops.DveOp(
        "SQ_ADD_ANT",
        Spec(
            body=sq(Src0) + sq(Src1),
            reference=lambda in0, in1, s0, s1, imm2: (
                in0.astype(np.float32) ** 2 + in1.astype(np.float32) ** 2
            ),
        ),
        subdim=False,
        uops_sha={},
    )
    dve_ops.OPS.append(op)
    dve_ops.CUSTOM_DVE_SPECS[op.name] = op.spec
    dve_ops._SUB_OPCODE_FOR_NAME[op.name] = (
        dve_ops._CUSTOM_DVE_ROW_BASE + len(dve_ops.OPS) - 1
    )
    # pin the sha so DveOp.compile()'s drift check passes
    for ver in ("v3", "v4"):
        spec = DveOpSpec(
            name=op.name,
            opcode=dve_ops.get_dve_sub_opcode(op.name),
            uops=lower(op.spec, ver=ver),
            rd1_en=True,
        )
        op.uops_sha[ver] = spec.sha(ver)
    return op


def _build():
    import concourse.bass as bass
    import concourse.mybir as mybir
    import concourse.tile as tile

    f32 = mybir.dt.float32
    bf16 = mybir.dt.bfloat16
    AF = mybir.ActivationFunctionType
    ALU = mybir.AluOpType

    # knobs (defaults = best measured config)
    pool_mode = int(os.environ.get("GCDD_POOL", "0"))  # 0 none, 1 dP, 2 d+dP
    uxx_act_tiles = int(os.environ.get("GCDD_UXX_ACT", "5"))  # uxx evac on ACT for t < this
    out_act_tiles = int(os.environ.get("GCDD_OUT_ACT", "0"))  # out evac via idt-MM+ACT for t < this
    # v3 rebalance knobs: move PSUM->SBUF evacs to the idle Pool engine and
    # fold the 3-matmul Sy convs to 2 matmuls via a precomputed u\-+u\+ pass.
    evac_split = os.environ.get("GCDD_EVAC_SPLIT", "0") == "1"  # uy evac on Pool
    uxx_pool = os.environ.get("GCDD_UXX_POOL", "0") == "1"  # uxx evac on Pool
    uy_fold = os.environ.get("GCDD_UY_FOLD", "off")  # off|pool|dve: Sy(u) 3mm->2mm
    q_fold = os.environ.get("GCDD_Q_FOLD", "off")  # off|pool|dve: Sy(Q) 3mm->2mm
    s2_pool = os.environ.get("GCDD_S2_POOL", "0") == "1"  # s2 add on Pool
    sqxy_dve = os.environ.get("GCDD_SQXY_DVE", "0") == "1"  # uxy^2 as DVE tt(ps,ps)
    sqx_act = os.environ.get("GCDD_SQX_ACT", "0") == "1"  # ux^2 on ACT square
    psa_bufs = int(os.environ.get("GCDD_PSA_BUFS", "2"))
    psdiv_bufs = int(os.environ.get("GCDD_PSDIV_BUFS", "1"))
    sub_pool = os.environ.get("GCDD_SUB_POOL", "0") == "1"  # num sub on Pool
    ag_pool = os.environ.get("GCDD_AG_POOL", "0") == "1"  # aG mul on Pool
    q_pool = os.environ.get("GCDD_Q_POOL", "0") == "1"  # q stt on Pool
    abs_mode = os.environ.get("GCDD_ABS", "int")  # int | act | stt
    sq_fuse = os.environ.get("GCDD_SQFUSE", "0") == "1"  # ux^2+uy^2 custom op
    sq_op = _sq_add_op() if sq_fuse else None
    skew = os.environ.get("GCDD_SKEW", "0") == "1"  # delay stage C one image

    nc = bass.Bass()
    u_dram = nc.dram_tensor("u", [IMGS, H, W], bf16, kind="ExternalInput")
    bands_dram = nc.dram_tensor("bands", [6, 128, 128], bf16, kind="ExternalInput")
    out_dram = nc.dram_tensor("out", [IMGS, H, W], bf16, kind="ExternalOutput")

    with tile.TileContext(nc) as tc:
        with (
            tc.tile_pool(name="const", bufs=1) as cpool,
            tc.tile_pool(name="pad", bufs=1) as ppool,
            tc.tile_pool(name="pad2", bufs=2) as ppool2,
            tc.tile_pool(
                name="upad", bufs=3 if os.environ.get("GCDD_SKEW", "0") == "1" else 2
            ) as upool,
            tc.tile_pool(name="dx", bufs=2) as dpool,
            tc.tile_pool(name="fold", bufs=2) as fpool,
            tc.tile_pool(name="plain2", bufs=2) as spool2,
            tc.tile_pool(name="psum_a", bufs=psa_bufs, space="PSUM") as qpool_a,
            tc.tile_pool(name="psum_b", bufs=1, space="PSUM") as qpool_b,
            tc.tile_pool(name="psum_d", bufs=psdiv_bufs, space="PSUM") as qpool_d,
        ):
            bsp = cpool.tile([128, 128], bf16, tag="bsp")
            bsm = cpool.tile([128, 128], bf16, tag="bsm")
            bd = cpool.tile([128, 128], bf16, tag="bd")
            bd2 = cpool.tile([128, 128], bf16, tag="bd2")
            idt = cpool.tile([128, 128], bf16, tag="idt")
            a2 = cpool.tile([128, 128], bf16, tag="a2")
            for j, b_ in enumerate((bsp, bsm, bd, bd2, idt, a2)):
                nc.sync.dma_start(out=b_[:], in_=bands_dram[j])

            ldw_order = os.environ.get("GCDD_LDW_ORDER", "0") == "1"

            def mm_sx(ps, src, t, start=True, stop=True):
                # conv columns of SOBEL_X: dx=-1 -> BSm, dx=+1 -> BSp
                base = BLK * t + PAD
                for j, (b_, dx) in enumerate(((bsm, -1), (bsp, +1))):
                    nc.tensor.matmul(
                        ps[:],
                        b_[:],
                        src[:, base + dx : base + dx + W],
                        start=(j == 0) and start,
                        stop=(j == 1) and stop,
                    )

            def mm_sy(ps, src, t, start=True, stop=True, fold_e=None):
                # conv columns of SOBEL_Y: dx=-1 -> BD, 0 -> BD2, +1 -> BD.
                # ldw_order: issue the two BD taps back-to-back so walrus's
                # ldw-opt can skip the second weight load.
                # fold_e: precomputed src[-1]+src[+1]; Sy = BD@fold_e + BD2@src
                # (2 matmuls instead of 3).
                base = BLK * t + PAD
                if fold_e is not None:
                    nc.tensor.matmul(
                        ps[:], bd[:], fold_e[:, base : base + W],
                        start=start, stop=False,
                    )
                    nc.tensor.matmul(
                        ps[:], bd2[:], src[:, base : base + W],
                        start=False, stop=stop,
                    )
                    return
                taps = (
                    ((bd, -1, True, False), (bd, +1, False, False), (bd2, 0, False, True))
                    if ldw_order
                    else ((bd, -1, True, False), (bd2, 0, False, False), (bd, +1, False, True))
                )
                for b_, dx, st_, sp_ in taps:
                    nc.tensor.matmul(
                        ps[:],
                        b_[:],
                        src[:, base + dx : base + dx + W],
                        start=st_ and start,
                        stop=sp_ and stop,
                    )

            import contextlib
            reps = int(os.environ.get("GCDD_REPS", "0"))
            # Unroll batch passes inside the hardware loop: For_i barriers all
            # engines each iteration, so the ~20us pipeline ramp+drain is paid
            # per trip; unrolling amortizes it over `unroll` full batch passes.
            # GCDD_REPS still counts batch passes (trips = reps // unroll).
            unroll = int(os.environ.get("GCDD_UNROLL", "3")) if reps > 1 else 1
            if reps > 1:
                unroll = max(1, min(unroll, reps))
                while reps % unroll:
                    unroll -= 1
                loop_cm = tc.For_i(0, reps // unroll)
            else:
                loop_cm = contextlib.nullcontext()
            def emit_u_load(img, alloc_idx):
                """Allocate u_pad for image `img` and emit its DMA loads.
                Called one image EARLY (at the top of the previous image's
                body) so loads clear the DMA queue before stage_a needs them."""
                u_pad = upool.tile([128, WIDTH], bf16, tag="u")
                if alloc_idx < (3 if skew else 2):
                    v = u_pad[:].rearrange("p (n b) -> p n b", b=BLK)
                    nc.vector.memset(v[:, :, 0:PAD], 0)
                    nc.vector.memset(v[:, :, PAD + W : BLK], 0)
                for t in range(NT):
                    st = TILE_STARTS[t]
                    nc.sync.dma_start(
                        out=u_pad[:, BLK * t + PAD : BLK * t + PAD + W],
                        in_=u_dram[img, st : st + 128, :],
                    )
                return {"u_pad": u_pad, "e_u": None}

            def emit_e_u(pf):
                """Emit the folded-Sy e_u = u[-1]+u[+1] pass for a prefetched
                image. Deferred to mid-body of the previous image so it sits
                behind that image's pointwise work in the engine queue (the
                DMA it waits on is already done), not at the queue head."""
                if uy_fold == "off" or pf is None or pf["e_u"] is not None:
                    return
                e_nx = fpool.tile([128, WIDTH], bf16, tag="e_u")
                u_nx = pf["u_pad"]
                e_eng = nc.gpsimd if uy_fold == "pool" else nc.vector
                e_eng.tensor_add(
                    e_nx[:, 1 : WIDTH - 1],
                    u_nx[:, 0 : WIDTH - 2],
                    u_nx[:, 2:WIDTH],
                )
                pf["e_u"] = e_nx

            with loop_cm:
              prev_c = [None]
              flat = [(u_, i) for u_ in range(unroll) for i in range(IMGS)]
              pending = [None]
              for idx in range(len(flat)):
               u_, i = flat[idx]
               if True:
                if pending[0] is None:
                    # first image of the For_i body: load + e_u inline
                    pending[0] = emit_u_load(i, idx)
                    emit_e_u(pending[0])
                u_pad = pending[0]["u_pad"]
                e_u = pending[0]["e_u"]
                pending[0] = (
                    emit_u_load(flat[idx + 1][1], idx + 1)
                    if idx + 1 < len(flat)
                    else None
                )
                uxuy_pad = ppool2.tile([128, 2 * WIDTH], bf16, tag="uxuy")
                pq_pad = (ppool2 if skew else ppool).tile(
                    [128, 2 * WIDTH], bf16, tag="pq"
                )
                uxxs = spool2.tile([128, PWIDTH], bf16, tag="uxxs")
                sqxy = spool2.tile([128, PWIDTH], bf16, tag="sqxy")
                ta = spool2.tile([128, PWIDTH], bf16, tag="ta")
                tb = spool2.tile([128, PWIDTH], bf16, tag="tb")
                tnum = spool2.tile([128, PWIDTH], bf16, tag="tnum")
                outs = spool2.tile([128, PWIDTH], bf16, tag="outs")
                if pool_mode >= 3:
                    dxu = dpool.tile([128, WIDTH], bf16, tag="dxu")
                    dxd2 = dpool.tile([128, PWIDTH], bf16, tag="dxd2")
                elif pool_mode >= 2:
                    dxu = dpool.tile([128, PWIDTH], bf16, tag="dxu")
                if pool_mode >= 1:
                    dxp = dpool.tile([128, PWIDTH], bf16, tag="dxp")

                # zero the x-halo pad columns of every shifted-read tensor
                # (pads are never overwritten afterwards, so only fresh pool
                # slots need it: first `bufs` images per tag)
                fr = u_ == 0
                fresh2 = (uxuy_pad,) if (fr and i < 2) else ()
                fresh2 += (dxu,) if (pool_mode >= 3 and fr and i < 2) else ()
                fresh1 = (pq_pad,) if (fr and (i == 0 or (skew and i == 1))) else ()
                for t_ in fresh2 + fresh1:
                    v = t_[:].rearrange("p (n b) -> p n b", b=BLK)
                    nc.vector.memset(v[:, :, 0:PAD], 0)
                    nc.vector.memset(v[:, :, PAD + W : BLK], 0)

                # 3D views
                u3 = u_pad[:].rearrange("p (n b) -> p n b", b=BLK)
                uc = u3[:, :, PAD : PAD + W]
                uxuyv = uxuy_pad[:].rearrange("p (m n b) -> p m n b", m=2, b=BLK)
                uxuyc = uxuyv[:, :, :, PAD : PAD + W]
                ux3 = uxuy_pad[:, :WIDTH].rearrange("p (n b) -> p n b", b=BLK)
                uxc = ux3[:, :, PAD : PAD + W]
                uy3 = uxuy_pad[:, WIDTH:].rearrange("p (n b) -> p n b", b=BLK)
                uyc = uy3[:, :, PAD : PAD + W]
                pq3 = pq_pad[:].rearrange("p (m n b) -> p m n b", m=2, b=BLK)
                pqc = pq3[:, :, :, PAD : PAD + W]
                p3 = pq_pad[:, :WIDTH].rearrange("p (n b) -> p n b", b=BLK)
                uxx3 = uxxs[:].rearrange("p (n b) -> p n b", b=W)
                sqxy3 = sqxy[:].rearrange("p (n b) -> p n b", b=W)
                ta3 = ta[:].rearrange("p (n b) -> p n b", b=W)
                tb3 = tb[:].rearrange("p (n b) -> p n b", b=W)
                tnum3 = tnum[:].rearrange("p (n b) -> p n b", b=W)
                outs3 = outs[:].rearrange("p (n b) -> p n b", b=W)
                if pool_mode >= 3:
                    dxu3 = dxu[:].rearrange("p (n b) -> p n b", b=BLK)[:, :, PAD : PAD + W]
                    dxd23 = dxd2[:].rearrange("p (n b) -> p n b", b=W)
                elif pool_mode >= 2:
                    dxu3 = dxu[:].rearrange("p (n b) -> p n b", b=W)
                if pool_mode >= 1:
                    dxp3 = dxp[:].rearrange("p (n b) -> p n b", b=W)

                e_q = None
                if q_fold != "off":
                    e_q = fpool.tile([128, WIDTH], bf16, tag="e_q")

                # ---- stage A: first derivatives -------------------------
                def stage_a(t):
                    base = BLK * t + PAD
                    ps_a = qpool_a.tile([128, 2 * W], f32, tag="ps_a")
                    if pool_mode >= 2:
                        # d = Dx(u) on Pool; ux = A @ d (1 matmul)
                        nc.gpsimd.tensor_sub(
                            dxu3[:, t, :],
                            u_pad[:, base + 1 : base + 1 + W],
                            u_pad[:, base - 1 : base - 1 + W],
                        )
                        nc.tensor.matmul(
                            ps_a[:, :W], bsp[:], dxu3[:, t, :], start=True, stop=True
                        )
                    else:
                        mm_sx(ps_a[:, :W], u_pad, t)
                    mm_sy(ps_a[:, W:], u_pad, t, fold_e=e_u)
                    if evac_split:
                        nc.scalar.copy(uxuyc[:, 0, t, :], ps_a[:, :W])
                        nc.gpsimd.tensor_copy(uxuyc[:, 1, t, :], ps_a[:, W:])
                    else:
                        nc.scalar.copy(
                            uxuyc[:, :, t, :],
                            ps_a[:].rearrange("p (m w) -> p m w", m=2),
                        )

                # ---- stage B: second derivatives ------------------------
                def stage_b(t):
                    base = BLK * t + PAD
                    ps_uxx = qpool_b.tile([128, W], f32, tag="ps_uxx")
                    if pool_mode >= 3:
                        # d2 = Dx(d) on Pool; uxx = A^2 @ d2 (1 matmul)
                        nc.gpsimd.tensor_sub(
                            dxd23[:, t, :],
                            dxu[:, base + 1 : base + 1 + W],
                            dxu[:, base - 1 : base - 1 + W],
                        )
                        nc.tensor.matmul(
                            ps_uxx[:], a2[:], dxd23[:, t, :], start=True, stop=True
                        )
                    else:
                        mm_sx(ps_uxx, uxuy_pad[:, :WIDTH], t)
                    if uxx_pool:
                        nc.gpsimd.tensor_copy(uxx3[:, t, :], ps_uxx[:])
                    elif t < uxx_act_tiles:
                        nc.scalar.copy(uxx3[:, t, :], ps_uxx[:])
                    else:
                        nc.vector.tensor_copy(uxx3[:, t, :], ps_uxx[:])
                    ps_uxy = qpool_b.tile([128, W], f32, tag="ps_uxy")
                    mm_sy(ps_uxy, uxuy_pad[:, :WIDTH], t)
                    if sqxy_dve:
                        nc.vector.tensor_mul(sqxy3[:, t, :], ps_uxy[:], ps_uxy[:])
                    else:
                        nc.scalar.square(sqxy3[:, t, :], ps_uxy[:])
                    ps_uyy = qpool_b.tile([128, W], f32, tag="ps_uyy")
                    mm_sy(ps_uyy, uxuy_pad[:, WIDTH:], t)
                    # nm = uxx * uyy (one PSUM operand max per DVE op)
                    nc.vector.tensor_mul(tnum3[:, t, :], ps_uyy[:], uxx3[:, t, :])

                # ---- pointwise chain (chunked so DVE/ACT pipeline) ------
                def pointwise(lo, hi):
                    s = (slice(None), slice(lo, hi), slice(None))
                    if sq_fuse:
                        for t_ in range(lo, hi):  # s2 = ux^2 + uy^2, one inst
                            nc.vector._custom_dve(  # (rank-2 APs per tile)
                                sq_op,
                                out=ta3[:, t_, :],
                                in0=uxc[:, t_, :],
                                in1=uyc[:, t_, :],
                            )
                    else:
                        if sqx_act:
                            nc.scalar.square(ta3[s], uxc[s])  # ux^2 on ACT
                        else:
                            nc.vector.tensor_mul(ta3[s], uxc[s], uxc[s])  # ux^2
                        nc.vector.tensor_mul(tb3[s], uyc[s], uyc[s])  # uy^2
                        (nc.gpsimd if q_pool else nc.vector).tensor_add(
                            ta3[s], ta3[s], tb3[s]
                        )  # s2 = ux^2 + uy^2
                    # 1/q^2 = exp(-2 ln(s2 + 1)); the +1 rides Ln's free bias.
                    # Ln/Exp share one ACT table set.
                    nc.scalar.activation(ta3[s], ta3[s], AF.Ln, bias=1.0)
                    nc.scalar.activation(tb3[s], ta3[s], AF.Exp, scale=-2.0)
                    (nc.gpsimd if sub_pool else nc.vector).tensor_sub(
                        tnum3[s], tnum3[s], sqxy3[s]
                    )  # num = uxx*uyy - uxy^2
                    if abs_mode == "int":  # |num|: clear bf16 sign bits
                        iv = tnum3[s].bitcast(mybir.dt.int32)
                        nc.vector.tensor_scalar(
                            iv, iv, 0x7FFF7FFF, None, ALU.bitwise_and
                        )
                    elif abs_mode == "act":
                        nc.scalar.activation(tnum3[s], tnum3[s], AF.Abs)
                    else:  # |num| = max(-num, num)
                        nc.vector.scalar_tensor_tensor(
                            tnum3[s], tnum3[s], -1.0, tnum3[s], ALU.mult, ALU.max
                        )
                    (nc.gpsimd if ag_pool else nc.vector).tensor_mul(
                        tnum3[s], tnum3[s], tb3[s]
                    )  # aG
                    nc.scalar.activation(  # phi = exp(-aG)
                        tnum3[s], tnum3[s], AF.Exp, scale=-1.0
                    )
                    # P|Q = phi * (ux|uy) in one op (phi broadcast over m)
                    sm = (slice(None), slice(None), slice(lo, hi), slice(None))
                    nc.vector.tensor_mul(
                        pqc[sm],
                        tnum3[s].unsqueeze(1).broadcast_to((128, 2, hi - lo, W)),
                        uxuyc[sm],
                    )
                    if q_fold != "off":
                        # e_q = Q[-1] + Q[+1] for this chunk's blocks
                        qsrc = pq_pad[:, WIDTH:]
                        a, b = BLK * lo, BLK * hi
                        eng = nc.gpsimd if q_fold == "pool" else nc.vector
                        eng.tensor_add(
                            e_q[:, a + 1 : b - 1], qsrc[:, a : b - 2], qsrc[:, a + 2 : b]
                        )

                # ---- stage C: divergence + residual ---------------------
                # (state bound via default args: the saved closure must keep
                # THIS image's tiles when called during the next image)
                def stage_c(
                    t,
                    i=i,
                    pq_pad=pq_pad,
                    u_pad=u_pad,
                    uc=uc,
                    outs=outs,
                    outs3=outs3,
                    e_q=e_q,
                    dxp3=(dxp3 if pool_mode >= 1 else None),
                ):
                    base = BLK * t + PAD
                    ps_div = qpool_d.tile([128, W], f32, tag="ps_div")
                    if pool_mode >= 1:
                        nc.gpsimd.tensor_sub(  # dP = Dx(P) on Pool
                            dxp3[:, t, :],
                            pq_pad[:, base + 1 : base + 1 + W],
                            pq_pad[:, base - 1 : base - 1 + W],
                        )
                        nc.tensor.matmul(  # div += A @ dP
                            ps_div[:], bsp[:], dxp3[:, t, :], start=True, stop=False
                        )
                    else:
                        mm_sx(ps_div, pq_pad[:, :WIDTH], t, start=True, stop=False)
                    st = TILE_STARTS[t]
                    lo, hi = OUT_ROWS[t]
                    if t < out_act_tiles:
                        # residual via identity band in PSUM, evac on ACT
                        mm_sy(ps_div, pq_pad[:, WIDTH:], t, start=False, stop=False, fold_e=e_q)
                        nc.tensor.matmul(
                            ps_div[:],
                            idt[:],
                            u_pad[:, base : base + W],
                            start=False,
                            stop=True,
                        )
                        nc.scalar.copy(outs3[:, t, :], ps_div[:])
                    else:
                        mm_sy(ps_div, pq_pad[:, WIDTH:], t, start=False, stop=True, fold_e=e_q)
                        nc.vector.tensor_add(outs3[:, t, :], ps_div[:], uc[:, t, :])
                    nc.sync.dma_start(
                        out=out_dram[i, lo:hi, :],
                        in_=outs[lo - st : hi - st, W * t : W * t + W],
                    )

                import json as _json
                _ck = _json.loads(
                    os.environ.get("GCDD_CHUNKS", "[[0,1],[1,2],[2,4],[4,5]]")
                )
                if i == 0 and os.environ.get("GCDD_RAMP_CHUNKS", "0") == "1":
                    _ck = [[0, 1], [1, 2], [2, 3], [3, 4], [4, 5]]
                if (
                    i == IMGS - 1
                    and u_ == unroll - 1
                    and os.environ.get("GCDD_TAIL_CHUNKS", "1") == "1"
                ):
                    _ck = [[0, 1], [1, 2], [2, 3], [3, 4], [4, 5]]
                if os.environ.get("GCDD_ORDER", "stage") == "tile":
                    # per-tile interleave: shortens the pipeline ramp
                    ci = 0
                    for t in range(NT):
                        stage_a(t)
                        stage_b(t)
                        while ci < len(_ck) and _ck[ci][1] == t + 1:
                            pointwise(*_ck[ci])
                            if ci == 0:
                                emit_e_u(pending[0])
                            for tc_ in range(*_ck[ci]):
                                stage_c(tc_)
                            ci += 1
                elif skew:
                    # emit this image's A/B/pw, then LAST image's C: keeps
                    # the per-engine streams from stalling at image bounds
                    for t in range(NT):
                        stage_a(t)
                    for t in range(NT):
                        stage_b(t)
                    for ci_, (lo, hi) in enumerate(_ck):
                        pointwise(lo, hi)
                        if ci_ == 0:
                            emit_e_u(pending[0])
                    if prev_c[0] is not None:
                        pc = prev_c[0]
                        for t in range(NT):
                            pc(t)
                    prev_c[0] = stage_c
                else:
                    for t in range(NT):
                        stage_a(t)
                    for t in range(NT):
                        stage_b(t)
                    for ci_, (lo, hi) in enumerate(_ck):
                        pointwise(lo, hi)
                        if ci_ == 0:
                            emit_e_u(pending[0])
                    for t in range(NT):
                        stage_c(t)
              if skew and prev_c[0] is not None:
                for t in range(NT):
                    prev_c[0](t)

    _split_multiwaits(nc)
    return nc


def _get_nc():
    if "nc" not in _CACHE:
        _CACHE["nc"] = _build()
    return _CACHE["nc"]


def make_in_maps(u):
    import ml_dtypes

    u = np.ascontiguousarray(u, dtype=np.float32).astype(ml_dtypes.bfloat16)
    bands = _bands_np().astype(ml_dtypes.bfloat16)
    per = B // N_CORES
    return [
        {
            "u": u[i * per : (i + 1) * per].reshape(IMGS, H, W),
            "bands": bands,
        }
        for i in range(N_CORES)
    ]


def kernel(u: np.ndarray, theta: np.ndarray = None) -> np.ndarray:
    from concourse.bass_utils import run_bass_kernel_spmd

    nc = _get_nc()
    in_maps = make_in_maps(u)
    res = run_bass_kernel_spmd(
        nc,
        in_maps,
        core_ids=list(range(N_CORES)),
        trace=os.environ.get("GCDD_TRACE", "0") == "1",
    )
    _CACHE["last_result"] = res
    per = B // N_CORES
    out = np.empty((B, C, H, W), np.float32)
    for i in range(N_CORES):
        out[i * per : (i + 1) * per] = (
            res.results[i]["out"].astype(np.float32).reshape(per, C, H, W)
        )
    return out



# revision 17
# speedup vs baseline: 1.0330x; 1.0330x over previous
"""Trainium2 Bass kernel for nn_GCDDLayer (Gaussian-curvature diffusion layer).

Math (per 512x512 image, zero-padded 3x3 convs):
    ux  = conv(u, SOBEL_X);  uy  = conv(u, SOBEL_Y)
    uxx = conv(ux, SOBEL_X); uxy = conv(ux, SOBEL_Y); uyy = conv(uy, SOBEL_Y)
    G   = (uxx*uyy - uxy^2) / ((1 + ux^2 + uy^2)^2 + 1e-6)
    phi = exp(-|G|); P = phi*ux; Q = phi*uy
    out = u + conv(P, SOBEL_X) + conv(Q, SOBEL_Y)

Strategy: pure data parallel over batch (16 samples -> 8 cores x 2 samples),
each core processes 6 independent 512x512 images (2 samples x 3 channels),
each cut into 5 overlapping 128-row tiles (stride 122; 3-row halo absorbs the
3-deep conv chain). Convs run on the TensorEngine as banded-matrix matmuls
(y-direction via the band, x-direction via shifted column reads of zero-padded
SBUF tiles, accumulated in PSUM).

v2 (~140us vs v1's ~226us; measured via the GCDD_REPS slope method):
- bf16 for u, out, and all SBUF intermediates: 2x DVE tensor_tensor mode,
  halved DMA traffic. Verified vs the fp32 jax reference: rel err ~1.0e-2
  (budget 2e-2; fp32 v1 was ~1e-3).
- |num| as a bf16 sign-bit mask: bitcast to int32, tensor_scalar AND with
  0x7FFF7FFF (2x_2p) — replaces an ACT Abs pass.
- the +1 in q = 1+ux^2+uy^2 rides ACT Ln's free bias; the q reduction is a
  plain 2x tensor_add instead of a 1x scalar_tensor_tensor.
- engine balance (HW-swept): all PSUM->SBUF evacs of ux/uy/uxx/uxy on ACT,
  out = u + div adds on DVE, Ln/Exp/Exp on ACT (one table set), everything
  else pointwise on DVE. GPSIMD/Pool measured net-negative on HW for every
  offload tried (sim's 0.42-efficiency model is optimistic) - left off.
- pointwise chunked [[0,1],[1,2],[2,4],[4,5]] so ACT/DVE/PE pipeline within
  an image; the last image uses 1-tile chunks to shorten the drain tail.

Engine busy (CoreSim, per core): ACT ~118us, PE ~117us (540 matmuls), DVE
~108us -> three-way balanced; wall ~137us sim / ~141us HW. Rejected paths:
uxy from conv(uy,SX) (boundary error 5.7e-2), Pool x-derivative precompute,
custom fused DVE ops (walrus in this container can't codegen them), DMA from
PSUM (unsupported), emission skew/interleave (scheduler already optimal).
"""

import os

import numpy as np

B, C, H, W = 16, 3, 512, 512
N_CORES = 8
IMGS = (B // N_CORES) * C  # 6 images per core
# PAD=4 keeps the center views 4B-aligned (DVE 2x_1p needs aligned packed
# reads); 3 is the minimum the conv chain needs.
PAD = int(os.environ.get("GCDD_PAD", "4"))
BLK = W + 2 * PAD
NT = 5  # row tiles per image
TILE_STARTS = [0, 122, 244, 366, 384]
OUT_ROWS = [(0, 125), (125, 247), (247, 369), (369, 491), (491, 512)]
WIDTH = NT * BLK  # 2590
PWIDTH = NT * W  # 2560

_CACHE = {}


def _split_multiwaits(nc):
    """Walrus in this container accepts only one sync-wait per instruction;
    Tile emits multi-wait instructions. Split: for an instruction with k>1
    waits, insert k-1 single-wait NoOps before it on the same engine (engine
    queues are strict FIFO, so sequential waiting is equivalent)."""
    import concourse.mybir as mybir

    ctr = [0]

    def fresh(base):
        ctr[0] += 1
        return f"{base}-wsplit{ctr[0]}"

    for f in nc.m.functions:
        for b in f.blocks:
            changed = False
            newlist = []
            for ins in b.instructions:
                si = ins.sync_info
                if si is not None and len(si.on_wait) > 1:
                    waits = list(si.on_wait)
                    for w in waits[:-1]:
                        newlist.append(
                            mybir.InstNoOp(
                                name=fresh(ins.name),
                                engine=ins.engine,
                                debug=ins.debug,
                                ins=[],
                                outs=[],
                                sync_info=mybir.SyncInfo(on_wait=[w], on_update=[]),
                            )
                        )
                    ins.sync_info = mybir.SyncInfo(
                        on_wait=[waits[-1]], on_update=list(si.on_update)
                    )
                    changed = True
                newlist.append(ins)
            if changed:
                b.instructions = newlist


def _band(c0, c1, c2, n=128):
    # lhsT[k, m] = col[k - m + 1] (k: input row partition, m: output row)
    return (
        np.diag(np.full(n, c1))
        + np.diag(np.full(n - 1, c0), 1)
        + np.diag(np.full(n - 1, c2), -1)
    ).astype(np.float32)


def _bands_np():
    a = _band(1, 2, 1)
    return np.stack(
        [
            a,  # BSp: SOBEL_X col dx=+1 (also the A smoothing band)
            _band(-1, -2, -1),  # BSm: SOBEL_X col dx=-1
            _band(-1, 0, 1),  # BD : SOBEL_Y col dx=+-1
            _band(-2, 0, 2),  # BD2: SOBEL_Y col dx=0
            np.eye(128, dtype=np.float32),  # IDT: residual u
            (a @ a).astype(np.float32),  # A2: y-smooth twice (pentadiagonal)
        ]
    )


def _sq_add_op():
    """Custom DVE op: out = in0^2 + in1^2 (one instruction instead of
    mul+mul+add). Registered at runtime; the uop table ships in the NEFF."""
    import numpy as np

    import concourse.dve_ops as dve_ops
    from concourse.dve_spec import Spec, Src0, Src1, lower, sq
    from concourse.dve_table_gen import dve_ver_for
    from concourse.dve_uop import DveOpSpec

    for op in dve_ops.OPS:
        if op.name == "SQ_ADD_ANT":
            return op
    op = dve_ops.DveOp(
        "SQ_ADD_ANT",
        Spec(
            body=sq(Src0) + sq(Src1),
            reference=lambda in0, in1, s0, s1, imm2: (
                in0.astype(np.float32) ** 2 + in1.astype(np.float32) ** 2
            ),
        ),
        subdim=False,
        uops_sha={},
    )
    dve_ops.OPS.append(op)
    dve_ops.CUSTOM_DVE_SPECS[op.name] = op.spec
    dve_ops._SUB_OPCODE_FOR_NAME[op.name] = (
        dve_ops._CUSTOM_DVE_ROW_BASE + len(dve_ops.OPS) - 1
    )
    # pin the sha so DveOp.compile()'s drift check passes
    for ver in ("v3", "v4"):
        spec = DveOpSpec(
            name=op.name,
            opcode=dve_ops.get_dve_sub_opcode(op.name),
            uops=lower(op.spec, ver=ver),
            rd1_en=True,
        )
        op.uops_sha[ver] = spec.sha(ver)
    return op


def _build():
    import concourse.bass as bass
    import concourse.mybir as mybir
    import concourse.tile as tile

    f32 = mybir.dt.float32
    bf16 = mybir.dt.bfloat16
    AF = mybir.ActivationFunctionType
    ALU = mybir.AluOpType

    # knobs (defaults = best measured config)
    pool_mode = int(os.environ.get("GCDD_POOL", "0"))  # 0 none, 1 dP, 2 d+dP
    uxx_act_tiles = int(os.environ.get("GCDD_UXX_ACT", "5"))  # uxx evac on ACT for t < this
    out_act_tiles = int(os.environ.get("GCDD_OUT_ACT", "0"))  # out evac via idt-MM+ACT for t < this
    # v3 rebalance knobs: move PSUM->SBUF evacs to the idle Pool engine and
    # fold the 3-matmul Sy convs to 2 matmuls via a precomputed u\-+u\+ pass.
    evac_split = os.environ.get("GCDD_EVAC_SPLIT", "0") == "1"  # uy evac on Pool
    uxx_pool = os.environ.get("GCDD_UXX_POOL", "0") == "1"  # uxx evac on Pool
    uy_fold = os.environ.get("GCDD_UY_FOLD", "off")  # off|pool|dve: Sy(u) 3mm->2mm
    q_fold = os.environ.get("GCDD_Q_FOLD", "off")  # off|pool|dve: Sy(Q) 3mm->2mm
    s2_pool = os.environ.get("GCDD_S2_POOL", "0") == "1"  # s2 add on Pool
    sqxy_dve = os.environ.get("GCDD_SQXY_DVE", "0") == "1"  # uxy^2 as DVE tt(ps,ps)
    sqx_act = os.environ.get("GCDD_SQX_ACT", "0") == "1"  # ux^2 on ACT square
    psa_bufs = int(os.environ.get("GCDD_PSA_BUFS", "2"))
    psdiv_bufs = int(os.environ.get("GCDD_PSDIV_BUFS", "1"))
    sub_pool = os.environ.get("GCDD_SUB_POOL", "0") == "1"  # num sub on Pool
    ag_pool = os.environ.get("GCDD_AG_POOL", "0") == "1"  # aG mul on Pool
    q_pool = os.environ.get("GCDD_Q_POOL", "0") == "1"  # q stt on Pool
    abs_mode = os.environ.get("GCDD_ABS", "int")  # int | act | stt
    sq_fuse = os.environ.get("GCDD_SQFUSE", "0") == "1"  # ux^2+uy^2 custom op
    sq_op = _sq_add_op() if sq_fuse else None
    skew = os.environ.get("GCDD_SKEW", "0") == "1"  # delay stage C one image

    nc = bass.Bass()
    u_dram = nc.dram_tensor("u", [IMGS, H, W], bf16, kind="ExternalInput")
    bands_dram = nc.dram_tensor("bands", [6, 128, 128], bf16, kind="ExternalInput")
    out_dram = nc.dram_tensor("out", [IMGS, H, W], bf16, kind="ExternalOutput")

    with tile.TileContext(nc) as tc:
        with (
            tc.tile_pool(name="const", bufs=1) as cpool,
            tc.tile_pool(name="pad", bufs=1) as ppool,
            tc.tile_pool(name="pad2", bufs=2) as ppool2,
            tc.tile_pool(
                name="upad", bufs=3 if os.environ.get("GCDD_SKEW", "0") == "1" else 2
            ) as upool,
            tc.tile_pool(name="dx", bufs=2) as dpool,
            tc.tile_pool(name="fold", bufs=2) as fpool,
            tc.tile_pool(name="plain2", bufs=2) as spool2,
            tc.tile_pool(name="psum_a", bufs=psa_bufs, space="PSUM") as qpool_a,
            tc.tile_pool(name="psum_b", bufs=1, space="PSUM") as qpool_b,
            tc.tile_pool(name="psum_d", bufs=psdiv_bufs, space="PSUM") as qpool_d,
        ):
            bsp = cpool.tile([128, 128], bf16, tag="bsp")
            bsm = cpool.tile([128, 128], bf16, tag="bsm")
            bd = cpool.tile([128, 128], bf16, tag="bd")
            bd2 = cpool.tile([128, 128], bf16, tag="bd2")
            idt = cpool.tile([128, 128], bf16, tag="idt")
            a2 = cpool.tile([128, 128], bf16, tag="a2")
            for j, b_ in enumerate((bsp, bsm, bd, bd2, idt, a2)):
                nc.sync.dma_start(out=b_[:], in_=bands_dram[j])

            ldw_order = os.environ.get("GCDD_LDW_ORDER", "1") == "1"

            def mm_sx(ps, src, t, start=True, stop=True):
                # conv columns of SOBEL_X: dx=-1 -> BSm, dx=+1 -> BSp
                base = BLK * t + PAD
                for j, (b_, dx) in enumerate(((bsm, -1), (bsp, +1))):
                    nc.tensor.matmul(
                        ps[:],
                        b_[:],
                        src[:, base + dx : base + dx + W],
                        start=(j == 0) and start,
                        stop=(j == 1) and stop,
                    )

            def mm_sy(ps, src, t, start=True, stop=True, fold_e=None):
                # conv columns of SOBEL_Y: dx=-1 -> BD, 0 -> BD2, +1 -> BD.
                # ldw_order: issue the two BD taps back-to-back so walrus's
                # ldw-opt can skip the second weight load.
                # fold_e: precomputed src[-1]+src[+1]; Sy = BD@fold_e + BD2@src
                # (2 matmuls instead of 3).
                base = BLK * t + PAD
                if fold_e is not None:
                    nc.tensor.matmul(
                        ps[:], bd[:], fold_e[:, base : base + W],
                        start=start, stop=False,
                    )
                    nc.tensor.matmul(
                        ps[:], bd2[:], src[:, base : base + W],
                        start=False, stop=stop,
                    )
                    return
                taps = (
                    ((bd, -1, True, False), (bd, +1, False, False), (bd2, 0, False, True))
                    if ldw_order
                    else ((bd, -1, True, False), (bd2, 0, False, False), (bd, +1, False, True))
                )
                for b_, dx, st_, sp_ in taps:
                    nc.tensor.matmul(
                        ps[:],
                        b_[:],
                        src[:, base + dx : base + dx + W],
                        start=st_ and start,
                        stop=sp_ and stop,
                    )

            import contextlib
            reps = int(os.environ.get("GCDD_REPS", "0"))
            # Unroll batch passes inside the hardware loop: For_i barriers all
            # engines each iteration, so the ~20us pipeline ramp+drain is paid
            # per trip; unrolling amortizes it over `unroll` full batch passes.
            # GCDD_REPS still counts batch passes (trips = reps // unroll).
            unroll = int(os.environ.get("GCDD_UNROLL", "60")) if reps > 1 else 1
            if reps > 1:
                unroll = max(1, min(unroll, reps))
                while reps % unroll:
                    unroll -= 1
                loop_cm = tc.For_i(0, reps // unroll)
            else:
                loop_cm = contextlib.nullcontext()
            def emit_u_load(img, alloc_idx):
                """Allocate u_pad for image `img` and emit its DMA loads.
                Called one image EARLY (at the top of the previous image's
                body) so loads clear the DMA queue before stage_a needs them."""
                u_pad = upool.tile([128, WIDTH], bf16, tag="u")
                if alloc_idx < (3 if skew else 2):
                    v = u_pad[:].rearrange("p (n b) -> p n b", b=BLK)
                    nc.vector.memset(v[:, :, 0:PAD], 0)
                    nc.vector.memset(v[:, :, PAD + W : BLK], 0)
                for t in range(NT):
                    st = TILE_STARTS[t]
                    nc.sync.dma_start(
                        out=u_pad[:, BLK * t + PAD : BLK * t + PAD + W],
                        in_=u_dram[img, st : st + 128, :],
                    )
                return {"u_pad": u_pad, "e_u": None}

            def emit_e_u(pf):
                """Emit the folded-Sy e_u = u[-1]+u[+1] pass for a prefetched
                image. Deferred to mid-body of the previous image so it sits
                behind that image's pointwise work in the engine queue (the
                DMA it waits on is already done), not at the queue head."""
                if uy_fold == "off" or pf is None or pf["e_u"] is not None:
                    return
                e_nx = fpool.tile([128, WIDTH], bf16, tag="e_u")
                u_nx = pf["u_pad"]
                e_eng = nc.gpsimd if uy_fold == "pool" else nc.vector
                e_eng.tensor_add(
                    e_nx[:, 1 : WIDTH - 1],
                    u_nx[:, 0 : WIDTH - 2],
                    u_nx[:, 2:WIDTH],
                )
                pf["e_u"] = e_nx

            with loop_cm:
              prev_c = [None]
              flat = [(u_, i) for u_ in range(unroll) for i in range(IMGS)]
              pending = [None]
              for idx in range(len(flat)):
               u_, i = flat[idx]
               if True:
                if pending[0] is None:
                    # first image of the For_i body: load + e_u inline
                    pending[0] = emit_u_load(i, idx)
                    emit_e_u(pending[0])
                u_pad = pending[0]["u_pad"]
                e_u = pending[0]["e_u"]
                pending[0] = (
                    emit_u_load(flat[idx + 1][1], idx + 1)
                    if idx + 1 < len(flat)
                    else None
                )
                uxuy_pad = ppool2.tile([128, 2 * WIDTH], bf16, tag="uxuy")
                pq_pad = (ppool2 if skew else ppool).tile(
                    [128, 2 * WIDTH], bf16, tag="pq"
                )
                uxxs = spool2.tile([128, PWIDTH], bf16, tag="uxxs")
                sqxy = spool2.tile([128, PWIDTH], bf16, tag="sqxy")
                ta = spool2.tile([128, PWIDTH], bf16, tag="ta")
                tb = spool2.tile([128, PWIDTH], bf16, tag="tb")
                tnum = spool2.tile([128, PWIDTH], bf16, tag="tnum")
                outs = spool2.tile([128, PWIDTH], bf16, tag="outs")
                if pool_mode >= 3:
                    dxu = dpool.tile([128, WIDTH], bf16, tag="dxu")
                    dxd2 = dpool.tile([128, PWIDTH], bf16, tag="dxd2")
                elif pool_mode >= 2:
                    dxu = dpool.tile([128, PWIDTH], bf16, tag="dxu")
                if pool_mode >= 1:
                    dxp = dpool.tile([128, PWIDTH], bf16, tag="dxp")

                # zero the x-halo pad columns of every shifted-read tensor
                # (pads are never overwritten afterwards, so only fresh pool
                # slots need it: first `bufs` images per tag)
                fr = u_ == 0
                fresh2 = (uxuy_pad,) if (fr and i < 2) else ()
                fresh2 += (dxu,) if (pool_mode >= 3 and fr and i < 2) else ()
                fresh1 = (pq_pad,) if (fr and (i == 0 or (skew and i == 1))) else ()
                for t_ in fresh2 + fresh1:
                    v = t_[:].rearrange("p (n b) -> p n b", b=BLK)
                    nc.vector.memset(v[:, :, 0:PAD], 0)
                    nc.vector.memset(v[:, :, PAD + W : BLK], 0)

                # 3D views
                u3 = u_pad[:].rearrange("p (n b) -> p n b", b=BLK)
                uc = u3[:, :, PAD : PAD + W]
                uxuyv = uxuy_pad[:].rearrange("p (m n b) -> p m n b", m=2, b=BLK)
                uxuyc = uxuyv[:, :, :, PAD : PAD + W]
                ux3 = uxuy_pad[:, :WIDTH].rearrange("p (n b) -> p n b", b=BLK)
                uxc = ux3[:, :, PAD : PAD + W]
                uy3 = uxuy_pad[:, WIDTH:].rearrange("p (n b) -> p n b", b=BLK)
                uyc = uy3[:, :, PAD : PAD + W]
                pq3 = pq_pad[:].rearrange("p (m n b) -> p m n b", m=2, b=BLK)
                pqc = pq3[:, :, :, PAD : PAD + W]
                p3 = pq_pad[:, :WIDTH].rearrange("p (n b) -> p n b", b=BLK)
                uxx3 = uxxs[:].rearrange("p (n b) -> p n b", b=W)
                sqxy3 = sqxy[:].rearrange("p (n b) -> p n b", b=W)
                ta3 = ta[:].rearrange("p (n b) -> p n b", b=W)
                tb3 = tb[:].rearrange("p (n b) -> p n b", b=W)
                tnum3 = tnum[:].rearrange("p (n b) -> p n b", b=W)
                outs3 = outs[:].rearrange("p (n b) -> p n b", b=W)
                if pool_mode >= 3:
                    dxu3 = dxu[:].rearrange("p (n b) -> p n b", b=BLK)[:, :, PAD : PAD + W]
                    dxd23 = dxd2[:].rearrange("p (n b) -> p n b", b=W)
                elif pool_mode >= 2:
                    dxu3 = dxu[:].rearrange("p (n b) -> p n b", b=W)
                if pool_mode >= 1:
                    dxp3 = dxp[:].rearrange("p (n b) -> p n b", b=W)

                e_q = None
                if q_fold != "off":
                    e_q = fpool.tile([128, WIDTH], bf16, tag="e_q")

                # ---- stage A: first derivatives -------------------------
                def stage_a(t):
                    base = BLK * t + PAD
                    ps_a = qpool_a.tile([128, 2 * W], f32, tag="ps_a")
                    if pool_mode >= 2:
                        # d = Dx(u) on Pool; ux = A @ d (1 matmul)
                        nc.gpsimd.tensor_sub(
                            dxu3[:, t, :],
                            u_pad[:, base + 1 : base + 1 + W],
                            u_pad[:, base - 1 : base - 1 + W],
                        )
                        nc.tensor.matmul(
                            ps_a[:, :W], bsp[:], dxu3[:, t, :], start=True, stop=True
                        )
                    else:
                        mm_sx(ps_a[:, :W], u_pad, t)
                    mm_sy(ps_a[:, W:], u_pad, t, fold_e=e_u)
                    if evac_split:
                        nc.scalar.copy(uxuyc[:, 0, t, :], ps_a[:, :W])
                        nc.gpsimd.tensor_copy(uxuyc[:, 1, t, :], ps_a[:, W:])
                    else:
                        nc.scalar.copy(
                            uxuyc[:, :, t, :],
                            ps_a[:].rearrange("p (m w) -> p m w", m=2),
                        )

                # ---- stage B: second derivatives ------------------------
                def stage_b(t):
                    base = BLK * t + PAD
                    ps_uxx = qpool_b.tile([128, W], f32, tag="ps_uxx")
                    if pool_mode >= 3:
                        # d2 = Dx(d) on Pool; uxx = A^2 @ d2 (1 matmul)
                        nc.gpsimd.tensor_sub(
                            dxd23[:, t, :],
                            dxu[:, base + 1 : base + 1 + W],
                            dxu[:, base - 1 : base - 1 + W],
                        )
                        nc.tensor.matmul(
                            ps_uxx[:], a2[:], dxd23[:, t, :], start=True, stop=True
                        )
                    else:
                        mm_sx(ps_uxx, uxuy_pad[:, :WIDTH], t)
                    if uxx_pool:
                        nc.gpsimd.tensor_copy(uxx3[:, t, :], ps_uxx[:])
                    elif t < uxx_act_tiles:
                        nc.scalar.copy(uxx3[:, t, :], ps_uxx[:])
                    else:
                        nc.vector.tensor_copy(uxx3[:, t, :], ps_uxx[:])
                    ps_uxy = qpool_b.tile([128, W], f32, tag="ps_uxy")
                    mm_sy(ps_uxy, uxuy_pad[:, :WIDTH], t)
                    if sqxy_dve:
                        nc.vector.tensor_mul(sqxy3[:, t, :], ps_uxy[:], ps_uxy[:])
                    else:
                        nc.scalar.square(sqxy3[:, t, :], ps_uxy[:])
                    ps_uyy = qpool_b.tile([128, W], f32, tag="ps_uyy")
                    mm_sy(ps_uyy, uxuy_pad[:, WIDTH:], t)
                    # nm = uxx * uyy (one PSUM operand max per DVE op)
                    nc.vector.tensor_mul(tnum3[:, t, :], ps_uyy[:], uxx3[:, t, :])

                # ---- pointwise chain (chunked so DVE/ACT pipeline) ------
                def pointwise(lo, hi):
                    s = (slice(None), slice(lo, hi), slice(None))
                    if sq_fuse:
                        for t_ in range(lo, hi):  # s2 = ux^2 + uy^2, one inst
                            nc.vector._custom_dve(  # (rank-2 APs per tile)
                                sq_op,
                                out=ta3[:, t_, :],
                                in0=uxc[:, t_, :],
                                in1=uyc[:, t_, :],
                            )
                    else:
                        if sqx_act:
                            nc.scalar.square(ta3[s], uxc[s])  # ux^2 on ACT
                        else:
                            nc.vector.tensor_mul(ta3[s], uxc[s], uxc[s])  # ux^2
                        nc.vector.tensor_mul(tb3[s], uyc[s], uyc[s])  # uy^2
                        (nc.gpsimd if q_pool else nc.vector).tensor_add(
                            ta3[s], ta3[s], tb3[s]
                        )  # s2 = ux^2 + uy^2
                    # 1/q^2 = exp(-2 ln(s2 + 1)); the +1 rides Ln's free bias.
                    # Ln/Exp share one ACT table set.
                    nc.scalar.activation(ta3[s], ta3[s], AF.Ln, bias=1.0)
                    nc.scalar.activation(tb3[s], ta3[s], AF.Exp, scale=-2.0)
                    (nc.gpsimd if sub_pool else nc.vector).tensor_sub(
                        tnum3[s], tnum3[s], sqxy3[s]
                    )  # num = uxx*uyy - uxy^2
                    if abs_mode == "int":  # |num|: clear bf16 sign bits
                        iv = tnum3[s].bitcast(mybir.dt.int32)
                        nc.vector.tensor_scalar(
                            iv, iv, 0x7FFF7FFF, None, ALU.bitwise_and
                        )
                    elif abs_mode == "act":
                        nc.scalar.activation(tnum3[s], tnum3[s], AF.Abs)
                    else:  # |num| = max(-num, num)
                        nc.vector.scalar_tensor_tensor(
                            tnum3[s], tnum3[s], -1.0, tnum3[s], ALU.mult, ALU.max
                        )
                    (nc.gpsimd if ag_pool else nc.vector).tensor_mul(
                        tnum3[s], tnum3[s], tb3[s]
                    )  # aG
                    nc.scalar.activation(  # phi = exp(-aG)
                        tnum3[s], tnum3[s], AF.Exp, scale=-1.0
                    )
                    # P|Q = phi * (ux|uy) in one op (phi broadcast over m)
                    sm = (slice(None), slice(None), slice(lo, hi), slice(None))
                    nc.vector.tensor_mul(
                        pqc[sm],
                        tnum3[s].unsqueeze(1).broadcast_to((128, 2, hi - lo, W)),
                        uxuyc[sm],
                    )
                    if q_fold != "off":
                        # e_q = Q[-1] + Q[+1] for this chunk's blocks
                        qsrc = pq_pad[:, WIDTH:]
                        a, b = BLK * lo, BLK * hi
                        eng = nc.gpsimd if q_fold == "pool" else nc.vector
                        eng.tensor_add(
                            e_q[:, a + 1 : b - 1], qsrc[:, a : b - 2], qsrc[:, a + 2 : b]
                        )

                # ---- stage C: divergence + residual ---------------------
                # (state bound via default args: the saved closure must keep
                # THIS image's tiles when called during the next image)
                def stage_c(
                    t,
                    i=i,
                    pq_pad=pq_pad,
                    u_pad=u_pad,
                    uc=uc,
                    outs=outs,
                    outs3=outs3,
                    e_q=e_q,
                    dxp3=(dxp3 if pool_mode >= 1 else None),
                ):
                    base = BLK * t + PAD
                    ps_div = qpool_d.tile([128, W], f32, tag="ps_div")
                    if pool_mode >= 1:
                        nc.gpsimd.tensor_sub(  # dP = Dx(P) on Pool
                            dxp3[:, t, :],
                            pq_pad[:, base + 1 : base + 1 + W],
                            pq_pad[:, base - 1 : base - 1 + W],
                        )
                        nc.tensor.matmul(  # div += A @ dP
                            ps_div[:], bsp[:], dxp3[:, t, :], start=True, stop=False
                        )
                    else:
                        mm_sx(ps_div, pq_pad[:, :WIDTH], t, start=True, stop=False)
                    st = TILE_STARTS[t]
                    lo, hi = OUT_ROWS[t]
                    if t < out_act_tiles:
                        # residual via identity band in PSUM, evac on ACT
                        mm_sy(ps_div, pq_pad[:, WIDTH:], t, start=False, stop=False, fold_e=e_q)
                        nc.tensor.matmul(
                            ps_div[:],
                            idt[:],
                            u_pad[:, base : base + W],
                            start=False,
                            stop=True,
                        )
                        nc.scalar.copy(outs3[:, t, :], ps_div[:])
                    else:
                        mm_sy(ps_div, pq_pad[:, WIDTH:], t, start=False, stop=True, fold_e=e_q)
                        nc.vector.tensor_add(outs3[:, t, :], ps_div[:], uc[:, t, :])
                    nc.sync.dma_start(
                        out=out_dram[i, lo:hi, :],
                        in_=outs[lo - st : hi - st, W * t : W * t + W],
                    )

                import json as _json
                _ck = _json.loads(
                    os.environ.get("GCDD_CHUNKS", "[[0,1],[1,2],[2,4],[4,5]]")
                )
                if i == 0 and os.environ.get("GCDD_RAMP_CHUNKS", "0") == "1":
                    _ck = [[0, 1], [1, 2], [2, 3], [3, 4], [4, 5]]
                if (
                    i == IMGS - 1
                    and u_ == unroll - 1
                    and os.environ.get("GCDD_TAIL_CHUNKS", "1") == "1"
                ):
                    _ck = [[0, 1], [1, 2], [2, 3], [3, 4], [4, 5]]
                if os.environ.get("GCDD_ORDER", "stage") == "tile":
                    # per-tile interleave: shortens the pipeline ramp
                    ci = 0
                    for t in range(NT):
                        stage_a(t)
                        stage_b(t)
                        while ci < len(_ck) and _ck[ci][1] == t + 1:
                            pointwise(*_ck[ci])
                            if ci == 0:
                                emit_e_u(pending[0])
                            for tc_ in range(*_ck[ci]):
                                stage_c(tc_)
                            ci += 1
                elif skew:
                    # emit this image's A/B/pw, then LAST image's C: keeps
                    # the per-engine streams from stalling at image bounds
                    for t in range(NT):
                        stage_a(t)
                    for t in range(NT):
                        stage_b(t)
                    for ci_, (lo, hi) in enumerate(_ck):
                        pointwise(lo, hi)
                        if ci_ == 0:
                            emit_e_u(pending[0])
                    if prev_c[0] is not None:
                        pc = prev_c[0]
                        for t in range(NT):
                            pc(t)
                    prev_c[0] = stage_c
                else:
                    for t in range(NT):
                        stage_a(t)
                    for t in range(NT):
                        stage_b(t)
                    for ci_, (lo, hi) in enumerate(_ck):
                        pointwise(lo, hi)
                        if ci_ == 0:
                            emit_e_u(pending[0])
                    for t in range(NT):
                        stage_c(t)
              if skew and prev_c[0] is not None:
                for t in range(NT):
                    prev_c[0](t)

    _split_multiwaits(nc)
    return nc


def _get_nc():
    if "nc" not in _CACHE:
        _CACHE["nc"] = _build()
    return _CACHE["nc"]


def make_in_maps(u):
    import ml_dtypes

    u = np.ascontiguousarray(u, dtype=np.float32).astype(ml_dtypes.bfloat16)
    bands = _bands_np().astype(ml_dtypes.bfloat16)
    per = B // N_CORES
    return [
        {
            "u": u[i * per : (i + 1) * per].reshape(IMGS, H, W),
            "bands": bands,
        }
        for i in range(N_CORES)
    ]


def kernel(u: np.ndarray, theta: np.ndarray = None) -> np.ndarray:
    from concourse.bass_utils import run_bass_kernel_spmd

    nc = _get_nc()
    in_maps = make_in_maps(u)
    res = run_bass_kernel_spmd(
        nc,
        in_maps,
        core_ids=list(range(N_CORES)),
        trace=os.environ.get("GCDD_TRACE", "0") == "1",
    )
    _CACHE["last_result"] = res
    per = B // N_CORES
    out = np.empty((B, C, H, W), np.float32)
    for i in range(N_CORES):
        out[i * per : (i + 1) * per] = (
            res.results[i]["out"].astype(np.float32).reshape(per, C, H, W)
        )
    return out

